# revision 23
# baseline (speedup 1.0000x reference)
"""CrossAttention + residual + LayerNorm on 8 Trainium2 NeuronCores.

Reference computation (per batch b):
    q = x @ Wq + bq ; k = ctx @ Wk + bk ; v = ctx @ Wv + bv      (16 heads of 64)
    attn = softmax(q k^T / 8) ; out = attn @ v
    y = LayerNorm(out @ Wo + bo + x) * gamma + beta

Sharding: core c -> batch b = c//4, query rows [512*(c%4), 512*(c%4+1)).
Each core recomputes K/V projections for its batch (cheaper than any
intra-chip collective at the measured 30-60 GB/s collective bandwidth).

v2 design (PE-bound baseline at 679us; fp32r matmuls + PE transposes +
serialized LDWEIGHTS dominated):
  - Host pre-casts ctx/x to fp8e4m3 (packed as uint16 byte pairs) and ships
    weights in fp8 DoubleRow layout -> no on-device transposes (DMA crossbar
    transposes the packed fp8) and no weight-cast passes.
  - All projections (Q/K/V/out) and O = P@V run as fp8 DoubleRow matmuls:
    256-deep contraction, 0.5 cyc/row -> half the PE stream time of fp32r.
    Contraction feature order f = 256*dp + 2*p + c (c = byte lane of the
    uint16 transpose) on the moving side; weights are host-permuted to match.
  - S = K^T q stays bf16 (accuracy) with the two heads of a pair row-tiled
    at (0,0)/(64,0).
  - exp runs on ACT over 2-PSUM-bank groups, writing P directly in fp8 with
    a uniform exponent shift exp(S/8 - 4) that cancels in normalization
    (keeps fp8 under its 448 max; logits reach +-6.7).
  - Denominators via a ones-column in V_aug (row 64 of O); batched
    reciprocal [8,512] per half; ones-matmul broadcast; DVE normalize.

CPU sim of this exact quantization: rel_err 4.0e-3 (gate 2e-2).
"""

import numpy as np

import concourse.bacc as bacc
import concourse.bass as bass
import concourse.tile as tile
from concourse import mybir

F32 = mybir.dt.float32
BF16 = mybir.dt.bfloat16
F8 = mybir.dt.float8e4
U16 = mybir.dt.uint16
U8 = mybir.dt.uint8
AF = mybir.ActivationFunctionType
DR = mybir.MatmulPerfMode.DoubleRow

B = 2
N = 2048          # context length
D = 1024          # model dim
H = 16            # heads
HD = 64           # head dim
NQ = 512          # query rows per core
SCALE = HD ** -0.5
SIGMA = 4.0       # uniform exponent shift: P = exp(S*SCALE - SIGMA)
LOG2E = 1.4426950408889634
# heads whose exp runs on DVE via the Schraudolph bf16-bit trick:
# u16 bits = S*(log2e/8)*128 + (127-SIGMA)*128, bitcast bf16, i.e.
# P = 2^(S/8/ln2 - SIGMA) with linear-mantissa interpolation.  The bit
# pattern stays in bf16 normal range for any |S| < 700, so no clamp is
# needed.  Softmax normalizes per head, so the approximation's uniform
# per-head bias cancels; never mix engines within one head.
DVE_HEADS = frozenset({1, 3, 5, 7, 9, 11, 13})
# Q is pre-scaled by log2e at cast time, so S' = S*log2e.  DVE heads
# build P in bf16 bit space: u16 bits = S'*16 + 128*(127-SIGMA), bitcast
# bf16 = 2^(S'/8 - SIGMA) with linear-mantissa interpolation.  The bit
# pattern stays in bf16 normal range for any |S'| < 7800 -> no clamp.
SCH_MUL = 16.0
SCH_ADD = 128.0 * (127.0 - SIGMA)
EPS = 1e-5

_CACHE = {}


def _emit(nc):
    with nc.allow_low_precision(reason="fp8/bf16 attention; validated vs fp32 sim"):
        _emit_body(nc)


def _emit_body(nc):
    xs = nc.dram_tensor("xs", [NQ, D], F32, kind="ExternalInput")
    xT8d = nc.dram_tensor("xT8d", [D, NQ], F8, kind="ExternalInput")
    ctxT8d = nc.dram_tensor("ctxT8d", [D, N], F8, kind="ExternalInput")
    wq8 = nc.dram_tensor("wq8", [4, 128, 2, D], F8, kind="ExternalInput")
    wk8 = nc.dram_tensor("wk8", [4, 128, 2, D], F8, kind="ExternalInput")
    wv8 = nc.dram_tensor("wv8", [4, 128, 2, D], F8, kind="ExternalInput")
    wo8 = nc.dram_tensor("wo8", [4, 128, 2, D], F8, kind="ExternalInput")
    bq = nc.dram_tensor("bq", [D], F32, kind="ExternalInput")
    bk = nc.dram_tensor("bk", [D], F32, kind="ExternalInput")
    bv = nc.dram_tensor("bv", [D], F32, kind="ExternalInput")
    bo = nc.dram_tensor("bo", [D], F32, kind="ExternalInput")
    gamma = nc.dram_tensor("gamma", [D], F32, kind="ExternalInput")
    beta = nc.dram_tensor("beta", [D], F32, kind="ExternalInput")
    y = nc.dram_tensor("y", [NQ, D], F32, kind="ExternalOutput")
    rscr = nc.dram_tensor("rscr", [H, NQ], F32, kind="Internal")

    def bcast_row(dram_vec):
        # [D] -> [128, D] DMA broadcast (partition step 0)
        a = dram_vec.ap()
        return bass.AP(tensor=a.tensor, offset=0, ap=[[0, 128]] + a.ap)

    def col_view(dram_vec):
        # [D] -> [128, 8] with [p, j] = vec[128*j + p]
        return dram_vec.ap().rearrange("(j p) -> p j", p=128)

    with tile.TileContext(nc) as tc, \
         tc.tile_pool(name="const", bufs=1) as const, \
         tc.tile_pool(name="inT", bufs=1) as inT, \
         tc.tile_pool(name="wts", bufs=4) as wpool, \
         tc.tile_pool(name="qk", bufs=1) as qk_pool, \
         tc.tile_pool(name="attn", bufs=1) as attn_pool:
        # ---- constants
        eps_t = const.tile([128, 1], F32)
        nc.vector.memset(eps_t, EPS)
        nsig_t = const.tile([128, 1], F32)
        nc.vector.memset(nsig_t, -SIGMA)
        bq_c = const.tile([128, 8], F32)
        nc.sync.dma_start(out=bq_c, in_=col_view(bq))
        bk_c = const.tile([128, 8], F32)
        nc.sync.dma_start(out=bk_c, in_=col_view(bk))
        bv_b = const.tile([128, D], F32)
        nc.sync.dma_start(out=bv_b, in_=bcast_row(bv))
        bo_b = const.tile([128, D], F32)
        nc.sync.dma_start(out=bo_b, in_=bcast_row(bo))
        gamma_b = const.tile([128, D], F32)
        nc.sync.dma_start(out=gamma_b, in_=bcast_row(gamma))
        beta_b = const.tile([128, D], F32)
        nc.sync.dma_start(out=beta_b, in_=bcast_row(beta))

        # ---- PE warm-up: ~4.5us of dummy matmuls during the input DMA
        # phase so the HAM clock gate reaches K=8/8 before real work.
        warm_w = const.tile([128, 128], F8)
        nc.vector.memset(warm_w, 0.25)
        warm_x = const.tile([128, NQ], F8)
        nc.vector.memset(warm_x, 0.25)
        with tc.tile_pool(name="pswarm", bufs=1, space="PSUM") as pswarm:
            wps = pswarm.tile([128, NQ], F32)
            for i in range(22):
                nc.tensor.matmul(wps, warm_w, warm_x,
                                 start=(i == 0), stop=(i == 21))

        # ---- fp8 transposed inputs (host pre-transposed): f = 128*j + p
        xT8 = inT.tile([128, 8, NQ], F8)
        nc.sync.dma_start(
            out=xT8, in_=xT8d.ap().rearrange("(j p) t -> p j t", p=128))
        ctxT8 = inT.tile([128, 8, N], F8)
        ctxr = ctxT8d.ap().rearrange("(j p) t -> p j t", p=128)
        for tc_ in range(4):
            nc.sync.dma_start(
                out=ctxT8[:, :, tc_ * 512:(tc_ + 1) * 512],
                in_=ctxr[:, :, tc_ * 512:(tc_ + 1) * 512])

        # ---- weights (fp8 DoubleRow layout, host-prepared)
        wq_t = [wpool.tile([128, 2, D], F8, name=f"wq{i}", tag="wq") for i in range(4)]
        wk_t = [wpool.tile([128, 2, D], F8, name=f"wk{i}", tag="wk") for i in range(4)]
        wv_t = [wpool.tile([128, 2, D], F8, name=f"wv{i}", tag="wv") for i in range(4)]
        wo_t = [wpool.tile([128, 2, D], F8, name=f"wo{i}", tag="wo") for i in range(4)]
        for dp in range(4):
            nc.scalar.dma_start(out=wq_t[dp], in_=wq8.ap()[dp])
        for dp in range(4):
            nc.scalar.dma_start(out=wk_t[dp], in_=wk8.ap()[dp])
        for dp in range(4):
            nc.scalar.dma_start(out=wv_t[dp], in_=wv8.ap()[dp])
        for dp in range(4):
            nc.scalar.dma_start(out=wo_t[dp], in_=wo8.ap()[dp])

        qT = qk_pool.tile([128, 8, NQ], BF16)      # Q^T, f = 128*fm + p
        kT = qk_pool.tile([128, 8, N], BF16)       # K^T
        vg = qk_pool.tile([128, 16, H, HD + 1], F8)  # V_aug: [tok, kt, h, hd+ones]
        nc.vector.memset(vg[:, :, :, HD:HD + 1], 1.0)

        attnT_raw = attn_pool.tile([128, 8, NQ], BF16)  # unnormalized O^T
        attnT8 = attn_pool.tile([128, 8, NQ], F8)       # normalized

        # ---- projections (fp8 DoubleRow, 256-deep contraction).
        # Q, K head-pair 0 and V heads 0-7 are computed up front; the rest
        # of K and V is emitted interleaved into the attention pair loop so
        # projection matmuls fill PE stalls (and casts fill ACT/DVE gaps).
        psp = ctx_psp = tc.tile_pool(name="psp", bufs=2, space="PSUM")
        psp = psp.__enter__()

        def emit_q(fm):
            pq = psp.tile([128, NQ], F32, tag="p")
            for dp in range(4):
                nc.tensor.matmul(
                    pq, wq_t[dp][:, :, fm * 128:(fm + 1) * 128],
                    xT8[:, 2 * dp:2 * dp + 2, :], start=(dp == 0), stop=(dp == 3),
                    perf_mode=DR,
                )
            nc.vector.tensor_scalar(
                out=qT[:, fm, :], in0=pq, scalar1=bq_c[:, fm:fm + 1],
                scalar2=LOG2E, op0=mybir.AluOpType.add,
                op1=mybir.AluOpType.mult,
            )

        def emit_k(fm, tn):
            pk = psp.tile([128, 512], F32, tag="p")
            for dp in range(4):
                nc.tensor.matmul(
                    pk, wk_t[dp][:, :, fm * 128:(fm + 1) * 128],
                    ctxT8[:, 2 * dp:2 * dp + 2, tn * 512:(tn + 1) * 512],
                    start=(dp == 0), stop=(dp == 3), perf_mode=DR,
                )
            nc.scalar.activation(
                out=kT[:, fm, tn * 512:(tn + 1) * 512], in_=pk,
                func=AF.Identity, bias=bk_c[:, fm:fm + 1], scale=1.0,
            )

        def emit_v(kt, hf):
            pv = psp.tile([128, 512], F32, tag="p")
            for dp in range(4):
                nc.tensor.matmul(
                    pv, ctxT8[:, 2 * dp:2 * dp + 2, kt * 128:(kt + 1) * 128],
                    wv_t[dp][:, :, hf * 512:(hf + 1) * 512],
                    start=(dp == 0), stop=(dp == 3), perf_mode=DR,
                )
            nc.vector.tensor_add(
                out=vg[:, kt, hf * 8:(hf + 1) * 8, 0:HD],
                in0=pv.rearrange("p (h c) -> p h c", h=8),
                in1=bv_b[:, hf * 512:(hf + 1) * 512].rearrange(
                    "p (h c) -> p h c", h=8),
            )

        for fm in range(8):
            emit_q(fm)
        for fm in range(8):
            for tn in range(4):
                emit_k(fm, tn)
        for kt in range(16):
            for hf in range(2):
                emit_v(kt, hf)

        work = {pj: [] for pj in range(8)}
        ctx_psp.__exit__(None, None, None)

        # ---- attention: one pair per iteration; head A exps on ACT,
        # head B on DVE (Schraudolph bf16-bit trick).  O for round kp is
        # issued during round kp+1 so the in-order PE queue never blocks
        # on a just-issued exp; deferred projection items drain between
        # rounds to fill the remaining PE gaps.
        with (
            tc.tile_pool(name="pp", bufs=8) as pp,
            tc.tile_pool(name="bs", bufs=4) as bs_pool,
            tc.tile_pool(name="sg", bufs=3, space="PSUM") as sg_pool,
            tc.tile_pool(name="po", bufs=2, space="PSUM") as po_pool,
        ):
            for pj in range(8):
                groups = [(pj, 0), (pj, 1)]
                items = list(work[pj])
                ot = {}
                for (pj_, l) in groups:
                    ot[(pj_, l)] = po_pool.tile(
                        [128, NQ], F32, name=f"o{pj_}_{l}", tag="o")

                def emit_o(kp, pX):
                    for (pj_, l) in groups:
                        h = 2 * pj_ + l
                        oX = ot[(pj_, l)]
                        if h in DVE_HEADS:
                            for i in range(2):
                                nc.tensor.matmul(
                                    oX[0:HD + 1, :],
                                    vg[:, 2 * kp + i, h, :],
                                    pX[(pj_, l)][:, i, :],
                                    start=(kp == 0 and i == 0),
                                    stop=(kp == 7 and i == 1),
                                )
                        else:
                            nc.tensor.matmul(
                                oX[0:HD + 1, :],
                                vg[:, 2 * kp:2 * kp + 2, h, :],
                                pX[(pj_, l)], start=(kp == 0), stop=(kp == 7),
                                perf_mode=DR,
                            )

                prev = None
                for kp in range(8):
                    pX = {}
                    for (pj_, l) in groups:
                        h = 2 * pj_ + l
                        sg = sg_pool.tile([128, 2, NQ], F32, tag="s")
                        for i in range(2):
                            kt = 2 * kp + i
                            ks = kT[:, pj_, kt * 128:(kt + 1) * 128]
                            nc.tensor.matmul(
                                sg[:, i, :], ks[64 * l:64 * l + 64],
                                qT[64 * l:64 * l + 64, pj_, :],
                                start=True, stop=True,
                                tile_position=(64 * l, 0),
                            )
                        if h in DVE_HEADS:
                            pU = pp.tile([128, 2, NQ], U16, tag="p16")
                            nc.vector.tensor_scalar(
                                out=pU, in0=sg, scalar1=SCH_MUL,
                                scalar2=SCH_ADD,
                                op0=mybir.AluOpType.mult,
                                op1=mybir.AluOpType.add)
                            pX[(pj_, l)] = pU.bitcast(BF16)
                        else:
                            pF = pp.tile([128, 2, NQ], F8, tag="p")
                            nc.scalar.activation(
                                out=pF, in_=sg, func=AF.Exp,
                                scale=float(SCALE / LOG2E), bias=nsig_t)
                            pX[(pj_, l)] = pF
                    if prev is not None:
                        emit_o(kp - 1, prev)
                    prev = pX
                    if items:
                        it = items.pop(0)
                        if it[0] == "k":
                            emit_k(it[1], it[2])
                        else:
                            emit_v(it[1], it[2])
                emit_o(7, prev)
                while items:
                    it = items.pop(0)
                    if it[0] == "k":
                        emit_k(it[1], it[2])
                    else:
                        emit_v(it[1], it[2])

                # epilogue: denominators -> 1/den -> DMA bounce broadcast
                bsf = {}
                for (pj_, l) in groups:
                    h = 2 * pj_ + l
                    oX = ot[(pj_, l)]
                    dsb = bs_pool.tile([1, NQ], F32, name=f"ds{pj_}_{l}", tag="ds")
                    nc.vector.tensor_copy(out=dsb, in_=oX[HD:HD + 1, :])
                    rf = bs_pool.tile([1, NQ], F32, name=f"rf{pj_}_{l}", tag="rf")
                    nc.vector.reciprocal_approx_fast(out=rf, in_=dsb)
                    nc.scalar.copy(
                        out=attnT_raw[64 * l:64 * l + 64, pj_, :], in_=oX[0:HD, :])
                    bsf[(pj_, l)] = rf
                bsb = bs_pool.tile([128, NQ], F32, name=f"bs{pj}", tag="bs")
                for l in range(2):
                    h = 2 * pj + l
                    nc.sync.dma_start(out=rscr.ap()[h], in_=bsf[(pj, l)])
                    a = rscr.ap()[h]
                    bc = bass.AP(tensor=a.tensor, offset=a.offset,
                                 ap=[[0, 64]] + list(a.ap))
                    nc.sync.dma_start(out=bsb[64 * l:64 * l + 64, :], in_=bc)
                nc.gpsimd.tensor_mul(
                    out=attnT8[:, pj, :], in0=attnT_raw[:, pj, :], in1=bsb)


        # ---- residual x + bo, token t = 128*i + p
        xbo = const.tile([128, 4, D], F32)
        nc.scalar.dma_start(out=xbo, in_=xs.ap().rearrange("(i p) d -> p i d", p=128))
        for i in range(4):
            nc.vector.tensor_add(out=xbo[:, i, :], in0=xbo[:, i, :], in1=bo_b)


        # ---- output projection (fp8 DoubleRow) + residual + LayerNorm
        with (
            tc.tile_pool(name="yb", bufs=2) as y_pool,
            tc.tile_pool(name="ln", bufs=4) as ln_pool,
            tc.tile_pool(name="psy", bufs=4, space="PSUM") as psy,
        ):
            yr = y.ap().rearrange("(i p) d -> p i d", p=128)
            for qm in range(4):
                ysb = y_pool.tile([128, D], F32, tag="y")
                for dn in range(2):
                    py = psy.tile([128, 512], F32, tag="y")
                    for op in range(4):
                        nc.tensor.matmul(
                            py, attnT8[:, 2 * op:2 * op + 2, qm * 128:(qm + 1) * 128],
                            wo_t[op][:, :, dn * 512:(dn + 1) * 512],
                            start=(op == 0), stop=(op == 3), perf_mode=DR,
                        )
                    nc.vector.tensor_add(
                        out=ysb[:, dn * 512:(dn + 1) * 512], in0=py,
                        in1=xbo[:, qm, dn * 512:(dn + 1) * 512],
                    )
                st = ln_pool.tile([128, 2, 6], F32, tag="st")
                for s2 in range(2):
                    nc.vector.bn_stats(out=st[:, s2, :], in_=ysb[:, s2 * 512:(s2 + 1) * 512])
                mv = ln_pool.tile([128, 2], F32, tag="mv")
                nc.vector.bn_aggr(out=mv, in_=st)
                nc.scalar.activation(
                    out=mv[:, 1:2], in_=mv[:, 1:2], func=AF.Sqrt, bias=eps_t, scale=1.0)
                nc.vector.reciprocal(out=mv[:, 1:2], in_=mv[:, 1:2])
                nc.vector.tensor_scalar(
                    out=ysb, in0=ysb, scalar1=mv[:, 0:1], scalar2=mv[:, 1:2],
                    op0=mybir.AluOpType.subtract, op1=mybir.AluOpType.mult,
                )
                nc.vector.tensor_mul(out=ysb, in0=ysb, in1=gamma_b)
                nc.vector.tensor_add(out=ysb, in0=ysb, in1=beta_b)
                nc.sync.dma_start(out=yr[:, qm, :], in_=ysb)

    return nc


def build():
    if "nc" not in _CACHE:
        nc = bacc.Bacc(trn_type="TRN2", target_bir_lowering=False, debug=False)
        _emit(nc)
        nc.compile()
        _CACHE["nc"] = nc
    return _CACHE["nc"]


def make_in_maps(x, context, Wq, bq, Wk, bk, Wv, bv, Wo, bo, gamma, beta):
    f32 = lambda a: np.ascontiguousarray(np.asarray(a, dtype=np.float32))
    f8np = mybir.dt.np(F8)

    def to8(a):
        return np.ascontiguousarray(np.asarray(a, np.float32).astype(f8np))

    x = f32(x)
    context = f32(context)
    # DoubleRow weight layout: [dp, p, s, fout] = W[256*dp + 128*s + p, fout]
    wdr = lambda W: np.ascontiguousarray(
        to8(W).reshape(4, 2, 128, D).transpose(0, 2, 1, 3))
    shared = {
        "wq8": wdr(Wq), "wk8": wdr(Wk), "wv8": wdr(Wv), "wo8": wdr(Wo),
        "bq": f32(bq), "bk": f32(bk), "bv": f32(bv), "bo": f32(bo),
        "gamma": f32(gamma), "beta": f32(beta),
    }
    ctxT8 = [np.ascontiguousarray(to8(context[b]).T) for b in range(B)]
    in_maps = []
    for c in range(8):
        b, qi = c // 4, c % 4
        m = dict(shared)
        xsl = x[b, qi * NQ:(qi + 1) * NQ, :]
        m["xs"] = np.ascontiguousarray(xsl)
        m["xT8d"] = np.ascontiguousarray(to8(xsl).T)
        m["ctxT8d"] = ctxT8[b]
        in_maps.append(m)
    return in_maps


def gather(results):
    y = np.empty((B, N, D), np.float32)
    for c in range(8):
        b, qi = c // 4, c % 4
        y[b, qi * NQ:(qi + 1) * NQ, :] = results[c]["y"]
    return y


def kernel(**inputs):
    from concourse import bass_utils

    nc = build()
    in_maps = make_in_maps(**inputs)
    res = bass_utils.run_bass_kernel_spmd(nc, in_maps, core_ids=list(range(8)))
    return gather(res.results)


# revision 24
# speedup vs baseline: 1.0224x; 1.0224x over previous
"""CrossAttention + residual + LayerNorm on 8 Trainium2 NeuronCores.

Reference computation (per batch b):
    q = x @ Wq + bq ; k = ctx @ Wk + bk ; v = ctx @ Wv + bv      (16 heads of 64)
    attn = softmax(q k^T / 8) ; out = attn @ v
    y = LayerNorm(out @ Wo + bo + x) * gamma + beta

Sharding: core c -> batch b = c//4, query rows [512*(c%4), 512*(c%4+1)).
Each core recomputes K/V projections for its batch (cheaper than any
intra-chip collective at the measured 30-60 GB/s collective bandwidth).

Design (baseline was fp32r everywhere at 679us, PE-bound with throttling):
  - Host pre-transposes and pre-casts: ctx^T/x^T shipped as fp8e4m3 DRAM
    tensors, weights shipped fp8 in DoubleRow layout -> no on-device
    transposes and no weight-cast passes.
  - All projections (Q/K/V/out) run as fp8 DoubleRow matmuls (256-deep
    contraction, 0.5 cyc/row).  Q is pre-scaled by log2e at its PSUM cast
    so S' = S*log2e comes out of the PE in log2 domain.
  - S = K^T q in bf16; the two heads of a pair are row-tiled at
    (0,0)/(64,0) and processed one pair per iteration with the exp of
    head A on ACT (exp table -> fp8 P, exponent shift 2^-SIGMA) and head
    B on DVE via a Schraudolph bit-trick (u16 bits = S'*16 + const,
    bitcast bf16 = 2^(S'/8 - SIGMA); the per-head uniform interpolation
    bias cancels in that head's own softmax normalization).  O matmuls
    are fp8 DoubleRow for ACT heads and mixed fp8xbf16 for DVE heads,
    issued one k-round late so the in-order PE queue never blocks on a
    just-issued exp.  A PE warm-up burst during the input DMAs plus the
    dense round pipeline keeps the HAM clock gate at K=8/8.
  - Denominators ride a ones-column in V_aug (row 64 of O PSUM);
    reciprocal_approx_fast (SBUF) + a DRAM-bounce partition-broadcast
    feed the gpsimd normalize multiply.
  - Residual + LayerNorm tail in fp32 on DVE; gamma/beta on DVE.

Measured: 254-258us on HW (2.6x over the fp32r baseline), rel_err 1.0e-2
(gate 2e-2); CPU sim of the quantization scheme alone: 4e-3.

"""

import numpy as np

import concourse.bacc as bacc
import concourse.bass as bass
import concourse.tile as tile
from concourse import mybir

F32 = mybir.dt.float32
BF16 = mybir.dt.bfloat16
F8 = mybir.dt.float8e4
U16 = mybir.dt.uint16
U8 = mybir.dt.uint8
AF = mybir.ActivationFunctionType
DR = mybir.MatmulPerfMode.DoubleRow

B = 2
N = 2048          # context length
D = 1024          # model dim
H = 16            # heads
HD = 64           # head dim
NQ = 512          # query rows per core
SCALE = HD ** -0.5
SIGMA = 4.0       # uniform exponent shift: P = exp(S*SCALE - SIGMA)
LOG2E = 1.4426950408889634
# heads whose exp runs on DVE via the Schraudolph bf16-bit trick:
# u16 bits = S*(log2e/8)*128 + (127-SIGMA)*128, bitcast bf16, i.e.
# P = 2^(S/8/ln2 - SIGMA) with linear-mantissa interpolation.  The bit
# pattern stays in bf16 normal range for any |S| < 700, so no clamp is
# needed.  Softmax normalizes per head, so the approximation's uniform
# per-head bias cancels; never mix engines within one head.
DVE_HEADS = frozenset({1, 3, 5, 7, 9, 11, 13})
# Q is pre-scaled by log2e at cast time, so S' = S*log2e.  DVE heads
# build P in bf16 bit space: u16 bits = S'*16 + 128*(127-SIGMA), bitcast
# bf16 = 2^(S'/8 - SIGMA) with linear-mantissa interpolation.  The bit
# pattern stays in bf16 normal range for any |S'| < 7800 -> no clamp.
SCH_MUL = 16.0
SCH_ADD = 128.0 * (127.0 - SIGMA)
EPS = 1e-5

_CACHE = {}


def _emit(nc):
    with nc.allow_low_precision(reason="fp8/bf16 attention; validated vs fp32 sim"):
        _emit_body(nc)


def _emit_body(nc):
    xs = nc.dram_tensor("xs", [NQ, D], F32, kind="ExternalInput")
    xT8d = nc.dram_tensor("xT8d", [D, NQ], F8, kind="ExternalInput")
    ctxT8d = nc.dram_tensor("ctxT8d", [D, N], F8, kind="ExternalInput")
    wq8 = nc.dram_tensor("wq8", [4, 128, 2, D], F8, kind="ExternalInput")
    wk8 = nc.dram_tensor("wk8", [4, 128, 2, D], F8, kind="ExternalInput")
    wv8 = nc.dram_tensor("wv8", [4, 128, 2, D], F8, kind="ExternalInput")
    wo8 = nc.dram_tensor("wo8", [4, 128, 2, D], F8, kind="ExternalInput")
    bq = nc.dram_tensor("bq", [D], F32, kind="ExternalInput")
    bk = nc.dram_tensor("bk", [D], F32, kind="ExternalInput")
    bv = nc.dram_tensor("bv", [D], F32, kind="ExternalInput")
    bo = nc.dram_tensor("bo", [D], F32, kind="ExternalInput")
    gamma = nc.dram_tensor("gamma", [D], F32, kind="ExternalInput")
    beta = nc.dram_tensor("beta", [D], F32, kind="ExternalInput")
    y = nc.dram_tensor("y", [NQ, D], F32, kind="ExternalOutput")
    rscr = nc.dram_tensor("rscr", [H, NQ], F32, kind="Internal")

    def bcast_row(dram_vec):
        # [D] -> [128, D] DMA broadcast (partition step 0)
        a = dram_vec.ap()
        return bass.AP(tensor=a.tensor, offset=0, ap=[[0, 128]] + a.ap)

    def col_view(dram_vec):
        # [D] -> [128, 8] with [p, j] = vec[128*j + p]
        return dram_vec.ap().rearrange("(j p) -> p j", p=128)

    with tile.TileContext(nc) as tc, \
         tc.tile_pool(name="const", bufs=1) as const, \
         tc.tile_pool(name="inT", bufs=1) as inT, \
         tc.tile_pool(name="wts", bufs=4) as wpool, \
         tc.tile_pool(name="qk", bufs=1) as qk_pool, \
         tc.tile_pool(name="attn", bufs=1) as attn_pool:
        # ---- constants
        eps_t = const.tile([128, 1], F32)
        nc.vector.memset(eps_t, EPS)
        nsig_t = const.tile([128, 1], F32)
        nc.vector.memset(nsig_t, -SIGMA)
        bq_c = const.tile([128, 8], F32)
        nc.sync.dma_start(out=bq_c, in_=col_view(bq))
        bk_c = const.tile([128, 8], F32)
        nc.sync.dma_start(out=bk_c, in_=col_view(bk))
        bv_b = const.tile([128, D], F32)
        nc.sync.dma_start(out=bv_b, in_=bcast_row(bv))
        bo_b = const.tile([128, D], F32)
        nc.sync.dma_start(out=bo_b, in_=bcast_row(bo))
        gamma_b = const.tile([128, D], F32)
        nc.sync.dma_start(out=gamma_b, in_=bcast_row(gamma))
        beta_b = const.tile([128, D], F32)
        nc.sync.dma_start(out=beta_b, in_=bcast_row(beta))

        # ---- PE warm-up: ~4.5us of dummy matmuls during the input DMA
        # phase so the HAM clock gate reaches K=8/8 before real work.
        warm_w = const.tile([128, 128], F8)
        nc.vector.memset(warm_w, 0.25)
        warm_x = const.tile([128, NQ], F8)
        nc.vector.memset(warm_x, 0.25)
        with tc.tile_pool(name="pswarm", bufs=1, space="PSUM") as pswarm:
            wps = pswarm.tile([128, NQ], F32)
            for i in range(22):
                nc.tensor.matmul(wps, warm_w, warm_x,
                                 start=(i == 0), stop=(i == 21))

        # ---- fp8 transposed inputs (host pre-transposed): f = 128*j + p
        xT8 = inT.tile([128, 8, NQ], F8)
        nc.sync.dma_start(
            out=xT8, in_=xT8d.ap().rearrange("(j p) t -> p j t", p=128))
        ctxT8 = inT.tile([128, 8, N], F8)
        ctxr = ctxT8d.ap().rearrange("(j p) t -> p j t", p=128)
        for tc_ in range(4):
            nc.sync.dma_start(
                out=ctxT8[:, :, tc_ * 512:(tc_ + 1) * 512],
                in_=ctxr[:, :, tc_ * 512:(tc_ + 1) * 512])

        # ---- weights (fp8 DoubleRow layout, host-prepared)
        wq_t = [wpool.tile([128, 2, D], F8, name=f"wq{i}", tag="wq") for i in range(4)]
        wk_t = [wpool.tile([128, 2, D], F8, name=f"wk{i}", tag="wk") for i in range(4)]
        wv_t = [wpool.tile([128, 2, D], F8, name=f"wv{i}", tag="wv") for i in range(4)]
        wo_t = [wpool.tile([128, 2, D], F8, name=f"wo{i}", tag="wo") for i in range(4)]
        for dp in range(4):
            nc.scalar.dma_start(out=wq_t[dp], in_=wq8.ap()[dp])
        for dp in range(4):
            nc.scalar.dma_start(out=wk_t[dp], in_=wk8.ap()[dp])
        for dp in range(4):
            nc.scalar.dma_start(out=wv_t[dp], in_=wv8.ap()[dp])
        for dp in range(4):
            nc.scalar.dma_start(out=wo_t[dp], in_=wo8.ap()[dp])

        qT = qk_pool.tile([128, 8, NQ], BF16)      # Q^T, f = 128*fm + p
        kT = qk_pool.tile([128, 8, N], BF16)       # K^T
        vg = qk_pool.tile([128, 16, H, HD + 1], F8)  # V_aug: [tok, kt, h, hd+ones]
        nc.vector.memset(vg[:, :, :, HD:HD + 1], 1.0)

        attnT_raw = attn_pool.tile([128, 8, NQ], BF16)  # unnormalized O^T
        attnT8 = attn_pool.tile([128, 8, NQ], F8)       # normalized

        # ---- projections (fp8 DoubleRow, 256-deep contraction).
        # Q, K head-pair 0 and V heads 0-7 are computed up front; the rest
        # of K and V is emitted interleaved into the attention pair loop so
        # projection matmuls fill PE stalls (and casts fill ACT/DVE gaps).
        psp = ctx_psp = tc.tile_pool(name="psp", bufs=2, space="PSUM")
        psp = psp.__enter__()

        def emit_q(fm):
            pq = psp.tile([128, NQ], F32, tag="p")
            for dp in range(4):
                nc.tensor.matmul(
                    pq, wq_t[dp][:, :, fm * 128:(fm + 1) * 128],
                    xT8[:, 2 * dp:2 * dp + 2, :], start=(dp == 0), stop=(dp == 3),
                    perf_mode=DR,
                )
            nc.vector.tensor_scalar(
                out=qT[:, fm, :], in0=pq, scalar1=bq_c[:, fm:fm + 1],
                scalar2=LOG2E, op0=mybir.AluOpType.add,
                op1=mybir.AluOpType.mult,
            )

        def emit_k(fm, tn):
            pk = psp.tile([128, 512], F32, tag="p")
            for dp in range(4):
                nc.tensor.matmul(
                    pk, wk_t[dp][:, :, fm * 128:(fm + 1) * 128],
                    ctxT8[:, 2 * dp:2 * dp + 2, tn * 512:(tn + 1) * 512],
                    start=(dp == 0), stop=(dp == 3), perf_mode=DR,
                )
            nc.scalar.activation(
                out=kT[:, fm, tn * 512:(tn + 1) * 512], in_=pk,
                func=AF.Identity, bias=bk_c[:, fm:fm + 1], scale=1.0,
            )

        def emit_v(kt, hf):
            pv = psp.tile([128, 512], F32, tag="p")
            for dp in range(4):
                nc.tensor.matmul(
                    pv, ctxT8[:, 2 * dp:2 * dp + 2, kt * 128:(kt + 1) * 128],
                    wv_t[dp][:, :, hf * 512:(hf + 1) * 512],
                    start=(dp == 0), stop=(dp == 3), perf_mode=DR,
                )
            nc.vector.tensor_add(
                out=vg[:, kt, hf * 8:(hf + 1) * 8, 0:HD],
                in0=pv.rearrange("p (h c) -> p h c", h=8),
                in1=bv_b[:, hf * 512:(hf + 1) * 512].rearrange(
                    "p (h c) -> p h c", h=8),
            )

        for fm in range(8):
            emit_q(fm)
        for fm in range(8):
            for tn in range(4):
                emit_k(fm, tn)
        for kt in range(16):
            for hf in range(2):
                emit_v(kt, hf)

        work = {pj: [] for pj in range(8)}
        ctx_psp.__exit__(None, None, None)

        # ---- attention: one pair per iteration; head A exps on ACT,
        # head B on DVE (Schraudolph bf16-bit trick).  O for round kp is
        # issued during round kp+1 so the in-order PE queue never blocks
        # on a just-issued exp; deferred projection items drain between
        # rounds to fill the remaining PE gaps.
        with (
            tc.tile_pool(name="pp", bufs=8) as pp,
            tc.tile_pool(name="bs", bufs=4) as bs_pool,
            tc.tile_pool(name="sg", bufs=3, space="PSUM") as sg_pool,
            tc.tile_pool(name="po", bufs=2, space="PSUM") as po_pool,
        ):
            for pj in range(8):
                groups = [(pj, 0), (pj, 1)]
                items = list(work[pj])
                ot = {}
                for (pj_, l) in groups:
                    ot[(pj_, l)] = po_pool.tile(
                        [128, NQ], F32, name=f"o{pj_}_{l}", tag="o")

                def emit_o(kp, pX):
                    for (pj_, l) in groups:
                        h = 2 * pj_ + l
                        oX = ot[(pj_, l)]
                        if h in DVE_HEADS:
                            for i in range(2):
                                nc.tensor.matmul(
                                    oX[0:HD + 1, :],
                                    vg[:, 2 * kp + i, h, :],
                                    pX[(pj_, l)][:, i, :],
                                    start=(kp == 0 and i == 0),
                                    stop=(kp == 7 and i == 1),
                                )
                        else:
                            nc.tensor.matmul(
                                oX[0:HD + 1, :],
                                vg[:, 2 * kp:2 * kp + 2, h, :],
                                pX[(pj_, l)], start=(kp == 0), stop=(kp == 7),
                                perf_mode=DR,
                            )

                prev = None
                for kp in range(8):
                    pX = {}
                    for (pj_, l) in groups:
                        h = 2 * pj_ + l
                        sg = sg_pool.tile([128, 2, NQ], F32, tag="s")
                        for i in range(2):
                            kt = 2 * kp + i
                            ks = kT[:, pj_, kt * 128:(kt + 1) * 128]
                            nc.tensor.matmul(
                                sg[:, i, :], ks[64 * l:64 * l + 64],
                                qT[64 * l:64 * l + 64, pj_, :],
                                start=True, stop=True,
                                tile_position=(64 * l, 0),
                            )
                        if h in DVE_HEADS:
                            pU = pp.tile([128, 2, NQ], U16, tag="p16")
                            nc.vector.tensor_scalar(
                                out=pU, in0=sg, scalar1=SCH_MUL,
                                scalar2=SCH_ADD,
                                op0=mybir.AluOpType.mult,
                                op1=mybir.AluOpType.add)
                            pX[(pj_, l)] = pU.bitcast(BF16)
                        else:
                            pF = pp.tile([128, 2, NQ], F8, tag="p")
                            nc.scalar.activation(
                                out=pF, in_=sg, func=AF.Exp,
                                scale=float(SCALE / LOG2E), bias=nsig_t)
                            pX[(pj_, l)] = pF
                    if prev is not None:
                        emit_o(kp - 1, prev)
                    prev = pX
                    if items:
                        it = items.pop(0)
                        if it[0] == "k":
                            emit_k(it[1], it[2])
                        else:
                            emit_v(it[1], it[2])
                emit_o(7, prev)
                while items:
                    it = items.pop(0)
                    if it[0] == "k":
                        emit_k(it[1], it[2])
                    else:
                        emit_v(it[1], it[2])

                # epilogue: denominators -> 1/den -> DMA bounce broadcast
                bsf = {}
                for (pj_, l) in groups:
                    h = 2 * pj_ + l
                    oX = ot[(pj_, l)]
                    dsb = bs_pool.tile([1, NQ], F32, name=f"ds{pj_}_{l}", tag="ds")
                    nc.vector.tensor_copy(out=dsb, in_=oX[HD:HD + 1, :])
                    rf = bs_pool.tile([1, NQ], F32, name=f"rf{pj_}_{l}", tag="rf")
                    nc.vector.reciprocal_approx_fast(out=rf, in_=dsb)
                    nc.scalar.copy(
                        out=attnT_raw[64 * l:64 * l + 64, pj_, :], in_=oX[0:HD, :])
                    bsf[(pj_, l)] = rf
                bsb = bs_pool.tile([128, NQ], F32, name=f"bs{pj}", tag="bs")
                for l in range(2):
                    h = 2 * pj + l
                    nc.sync.dma_start(out=rscr.ap()[h], in_=bsf[(pj, l)])
                    a = rscr.ap()[h]
                    bc = bass.AP(tensor=a.tensor, offset=a.offset,
                                 ap=[[0, 64]] + list(a.ap))
                    nc.sync.dma_start(out=bsb[64 * l:64 * l + 64, :], in_=bc)
                nc.gpsimd.tensor_mul(
                    out=attnT8[:, pj, :], in0=attnT_raw[:, pj, :], in1=bsb)


        # ---- residual x + bo, token t = 128*i + p
        xbo = const.tile([128, 4, D], F32)
        nc.scalar.dma_start(out=xbo, in_=xs.ap().rearrange("(i p) d -> p i d", p=128))
        for i in range(4):
            nc.vector.tensor_add(out=xbo[:, i, :], in0=xbo[:, i, :], in1=bo_b)


        # ---- output projection (fp8 DoubleRow) + residual + LayerNorm
        with (
            tc.tile_pool(name="yb", bufs=2) as y_pool,
            tc.tile_pool(name="ln", bufs=4) as ln_pool,
            tc.tile_pool(name="psy", bufs=4, space="PSUM") as psy,
        ):
            yr = y.ap().rearrange("(i p) d -> p i d", p=128)
            for qm in range(4):
                ysb = y_pool.tile([128, D], F32, tag="y")
                for dn in range(2):
                    py = psy.tile([128, 512], F32, tag="y")
                    for op in range(4):
                        nc.tensor.matmul(
                            py, attnT8[:, 2 * op:2 * op + 2, qm * 128:(qm + 1) * 128],
                            wo_t[op][:, :, dn * 512:(dn + 1) * 512],
                            start=(op == 0), stop=(op == 3), perf_mode=DR,
                        )
                    nc.vector.tensor_add(
                        out=ysb[:, dn * 512:(dn + 1) * 512], in0=py,
                        in1=xbo[:, qm, dn * 512:(dn + 1) * 512],
                    )
                st = ln_pool.tile([128, 2, 6], F32, tag="st")
                for s2 in range(2):
                    nc.vector.bn_stats(out=st[:, s2, :], in_=ysb[:, s2 * 512:(s2 + 1) * 512])
                mv = ln_pool.tile([128, 2], F32, tag="mv")
                nc.vector.bn_aggr(out=mv, in_=st)
                nc.scalar.activation(
                    out=mv[:, 1:2], in_=mv[:, 1:2], func=AF.Sqrt, bias=eps_t, scale=1.0)
                nc.vector.reciprocal(out=mv[:, 1:2], in_=mv[:, 1:2])
                nc.vector.tensor_scalar(
                    out=ysb, in0=ysb, scalar1=mv[:, 0:1], scalar2=mv[:, 1:2],
                    op0=mybir.AluOpType.subtract, op1=mybir.AluOpType.mult,
                )
                nc.vector.tensor_mul(out=ysb, in0=ysb, in1=gamma_b)
                nc.vector.tensor_add(out=ysb, in0=ysb, in1=beta_b)
                nc.sync.dma_start(out=yr[:, qm, :], in_=ysb)

    return nc


def build():
    if "nc" not in _CACHE:
        nc = bacc.Bacc(trn_type="TRN2", target_bir_lowering=False, debug=False)
        _emit(nc)
        nc.compile()
        _CACHE["nc"] = nc
    return _CACHE["nc"]


def make_in_maps(x, context, Wq, bq, Wk, bk, Wv, bv, Wo, bo, gamma, beta):
    f32 = lambda a: np.ascontiguousarray(np.asarray(a, dtype=np.float32))
    f8np = mybir.dt.np(F8)

    def to8(a):
        return np.ascontiguousarray(np.asarray(a, np.float32).astype(f8np))

    x = f32(x)
    context = f32(context)
    # DoubleRow weight layout: [dp, p, s, fout] = W[256*dp + 128*s + p, fout]
    wdr = lambda W: np.ascontiguousarray(
        to8(W).reshape(4, 2, 128, D).transpose(0, 2, 1, 3))
    shared = {
        "wq8": wdr(Wq), "wk8": wdr(Wk), "wv8": wdr(Wv), "wo8": wdr(Wo),
        "bq": f32(bq), "bk": f32(bk), "bv": f32(bv), "bo": f32(bo),
        "gamma": f32(gamma), "beta": f32(beta),
    }
    ctxT8 = [np.ascontiguousarray(to8(context[b]).T) for b in range(B)]
    in_maps = []
    for c in range(8):
        b, qi = c // 4, c % 4
        m = dict(shared)
        xsl = x[b, qi * NQ:(qi + 1) * NQ, :]
        m["xs"] = np.ascontiguousarray(xsl)
        m["xT8d"] = np.ascontiguousarray(to8(xsl).T)
        m["ctxT8d"] = ctxT8[b]
        in_maps.append(m)
    return in_maps


def gather(results):
    y = np.empty((B, N, D), np.float32)
    for c in range(8):
        b, qi = c // 4, c % 4
        y[b, qi * NQ:(qi + 1) * NQ, :] = results[c]["y"]
    return y


def kernel(**inputs):
    from concourse import bass_utils

    nc = build()
    in_maps = make_in_maps(**inputs)
    res = bass_utils.run_bass_kernel_spmd(nc, in_maps, core_ids=list(range(8)))
    return gather(res.results)
